# revision 1
# baseline (speedup 1.0000x reference)
"""Trainium2 kernel for nn_KernelEncodingLayer (von Mises kernel encoding).

Math
----
reference computes, per key n and bin b:
    logits[n,b] = sum_f mag[n,f] * sum_k w[b,f,k] * exp(kappa*(cos(angle[n,f]-mu_eff[b,f,k])-1))

The von Mises kernel expands exactly in a Fourier series (Bessel coefficients):
    exp(kappa*cos(d))*exp(-kappa) = e^-kappa * [I_0(kappa) + 2*sum_m I_m(kappa) cos(m d)]
Since kappa <= 1 the series converges superexponentially; truncating at m<=5 (cos)
/ m<=6 (sin) leaves ~5e-6 relative error.

With r = mag, u = cos(angle) = x/r, y = r*sin(angle):
    r*cos(m*angle) = sum_j chebT(m)[j] * (r * u^j)
    r*sin(m*angle) = sum_j chebU(m-1)[j] * (y * u^j)
so logits = sum_{f,j} P_j[b,f]*(r u^j)[n,f] + Q_j[b,f]*(y u^j)[n,f] + bias[b]
where P/Q fold Bessel values, Chebyshev coefficients, mu, kappa, weight and
reference_angles -- all tiny (b,f) arrays, computed on host in float64.

Device kernel (per core, 1024 keys):  features via a short DVE multiply chain,
then a [features x keys] @ [features x bins] PE matmul accumulated in PSUM,
bias added on PSUM->SBUF eviction, transposed output [bins, keys] DMA'd out.

Sharding: data-parallel over keys across 8 cores; weights replicated.
"""

import math

import numpy as np

import concourse.bacc as bacc
import concourse.bass as bass
import concourse.mybir as mybir
import concourse.tile as tile
from concourse._compat import with_exitstack
from concourse.bass_utils import run_bass_kernel_spmd
from concourse.mybir import AluOpType

# problem shape (hardcoded per harness contract)
NKEYS = 8192
NBINS = 128
NFREQ = 64
NCORES = 8
KPC = NKEYS // NCORES  # 1024 keys per core
FD = KPC  # free dim of on-chip tiles

NT = 6  # chain tiles T1..T6 -> cos harmonics m<=NT-1, sin harmonics m<=NT
NCHUNK = NT + 1  # matmul contraction chunks of 128 feature-rows

F32 = mybir.dt.float32


# ----------------------------------------------------------------------------
# host-side math: Bessel I_m and Chebyshev coefficient folding
# ----------------------------------------------------------------------------

def _bessel_i(m: int, x: np.ndarray) -> np.ndarray:
    x = np.asarray(x, np.float64)
    s = np.zeros_like(x)
    for j in range(24):
        s = s + (x / 2.0) ** (2 * j + m) / (math.factorial(j) * math.factorial(j + m))
    return s


def _cheb_t(m: int) -> np.ndarray:
    T = [np.array([1.0]), np.array([0.0, 1.0])]
    while len(T) <= m:
        a = np.zeros(len(T[-1]) + 1)
        a[1:] = 2 * T[-1]
        a[: len(T[-2])] -= T[-2]
        T.append(a)
    return T[m]


def _cheb_u(m: int) -> np.ndarray:
    U = [np.array([1.0]), np.array([0.0, 2.0])]
    while len(U) <= m:
        a = np.zeros(len(U[-1]) + 1)
        a[1:] = 2 * U[-1]
        a[: len(U[-2])] -= U[-2]
        U.append(a)
    return U[m]


def _build_device_weights(reference_angles, mu, kappa, weight) -> np.ndarray:
    """Fold everything bin/freq-dependent into [NCHUNK, 128, NBINS] fp32.

    Chunk row layout (contraction rows): rows 0:64 = top-half feature per
    freq, rows 64:128 = bottom-half feature per freq, matching the device
    feature chunks:
        chunk0 = [x; y]      -> (p1, q0)
        chunk1 = [r; y]      -> (p0, dup)
        chunk2 = [x; y*u]    -> (dup, q1)
        chunk k = [p_{k-1}; q_{k-1}]  for k = 3..NT
    """
    mu_eff = np.asarray(mu, np.float64) + np.asarray(reference_angles, np.float64)[None, :, None]
    kap = np.asarray(kappa, np.float64)
    w = np.asarray(weight, np.float64)

    P = np.zeros((NT, NBINS, NFREQ))  # coeff of p_j = r*u^j, j = 0..NT-1
    Q = np.zeros((NT, NBINS, NFREQ))  # coeff of q_j = y*u^j, j = 0..NT-1
    for m in range(0, NT):  # cos series m = 0..NT-1
        eps = 1.0 if m == 0 else 2.0
        coef = w * eps * _bessel_i(m, kap) * np.exp(-kap)
        A = (coef * np.cos(m * mu_eff)).sum(-1)  # (b, f)
        for j, c in enumerate(_cheb_t(m)):
            if c:
                P[j] += c * A
    for m in range(1, NT + 1):  # sin series m = 1..NT
        coef = w * 2.0 * _bessel_i(m, kap) * np.exp(-kap)
        B = (coef * np.sin(m * mu_eff)).sum(-1)
        for j, c in enumerate(_cheb_u(m - 1)):
            if c:
                Q[j] += c * B

    z = np.zeros((NFREQ, NBINS))
    W = np.zeros((NCHUNK, 2 * NFREQ, NBINS), np.float64)

    def top_bot(k, top, bot):
        W[k, :NFREQ] = top
        W[k, NFREQ:] = bot

    top_bot(0, P[1].T, Q[0].T)
    top_bot(1, P[0].T, z)
    top_bot(2, z, Q[1].T)
    for k in range(3, NCHUNK):
        top_bot(k, P[k - 1].T, Q[k - 1].T)
    return np.ascontiguousarray(W.astype(np.float32))


# ----------------------------------------------------------------------------
# device kernel
# ----------------------------------------------------------------------------

@with_exitstack
def _device_kernel(ctx, tc: tile.TileContext, out_d, xy_d, w_d, bias_d):
    nc = tc.nc
    const = ctx.enter_context(tc.tile_pool(name="const", bufs=1))
    work = ctx.enter_context(tc.tile_pool(name="work", bufs=1))
    psum = ctx.enter_context(tc.tile_pool(name="psum", bufs=1, space="PSUM"))

    xy = const.tile([128, FD], F32, tag="xy")
    nc.sync.dma_start(xy[:], xy_d[:])
    wt = []
    for k in range(NCHUNK):
        t = const.tile([128, NBINS], F32, tag=f"w{k}", name=f"w{k}")
        nc.sync.dma_start(t[:], w_d[k])
        wt.append(t)
    bias_t = const.tile([128, 1], F32, tag="bias")
    nc.sync.dma_start(bias_t[:], bias_d[:])

    HF = NFREQ  # 64: top half = x-derived, bottom half = y-derived

    # TT requires both inputs at the same base partition; ACT Square moves
    # y^2 down to base 0 on the otherwise-idle scalar engine.
    sq = work.tile([128, FD], F32, tag="sq")
    nc.vector.tensor_tensor(sq[:HF], xy[:HF], xy[:HF], AluOpType.mult)  # x^2
    syb = work.tile([128, FD], F32, tag="syb")
    nc.scalar.square(syb[:HF], xy[HF:])                                 # y^2 -> base 0
    r2 = work.tile([128, FD], F32, tag="r2")
    nc.vector.tensor_tensor(r2[:HF], sq[:HF], syb[:HF], AluOpType.add)

    T = [None] * (NT + 1)
    for k in range(1, NT + 1):
        T[k] = work.tile([128, FD], F32, tag=f"T{k}", name=f"T{k}")

    nc.scalar.sqrt(T[1][:HF], r2[:HF])          # r -> T1 top
    nc.scalar.copy(T[1][HF:], xy[HF:])          # y -> T1 bot

    ir = work.tile([128, FD], F32, tag="ir")
    nc.vector.reciprocal_approx_fast(out=ir[:HF], in_=T[1][:HF])  # 1/r (~18 bits)
    u2 = work.tile([128, FD], F32, tag="u2")
    nc.vector.tensor_tensor(u2[:HF], xy[:HF], ir[:HF], AluOpType.mult)  # u = x/r
    nc.scalar.copy(u2[HF:], u2[:HF])            # [u; u]

    for k in range(2, NT + 1):
        nc.vector.tensor_tensor(T[k][:], T[k - 1][:], u2[:], AluOpType.mult)

    chunks = [xy] + T[1:]
    H = FD // 2  # 512-key halves, one PSUM bank each
    ps = [psum.tile([128, H], F32, tag=f"ps{h}", name=f"ps{h}") for h in range(2)]
    for ci, ch in enumerate(chunks):
        for h in range(2):
            nc.tensor.matmul(
                ps[h][:],
                wt[ci][:],
                ch[:, h * H:(h + 1) * H],
                start=(ci == 0),
                stop=(ci == NCHUNK - 1),
            )

    osb = work.tile([128, FD], F32, tag="osb")
    for h in range(2):
        nc.scalar.add(osb[:, h * H:(h + 1) * H], ps[h][:], bias_t[:])
    nc.sync.dma_start(out_d[:], osb[:])


_COMPILED = None


def _get_compiled():
    global _COMPILED
    if _COMPILED is None:
        nc = bacc.Bacc("TRN2", target_bir_lowering=False, debug=False)
        xy = nc.dram_tensor("xy", [128, FD], F32, kind="ExternalInput").ap()
        w = nc.dram_tensor("w", [NCHUNK, 128, NBINS], F32, kind="ExternalInput").ap()
        b = nc.dram_tensor("bias", [NBINS, 1], F32, kind="ExternalInput").ap()
        out = nc.dram_tensor("out", [NBINS, FD], F32, kind="ExternalOutput").ap()
        with tile.TileContext(nc) as tc:
            _device_kernel(tc, out, xy, w, b)
        nc.compile()
        _COMPILED = nc
    return _COMPILED


# ----------------------------------------------------------------------------
# entry point
# ----------------------------------------------------------------------------

def _run(K, reference_angles, mu, kappa, weight, bias, **spmd_kwargs):
    K = np.ascontiguousarray(np.asarray(K, np.float32))
    x = K[:, 0::2]  # (NKEYS, NFREQ) real parts
    y = K[:, 1::2]  # imag parts

    in_maps = []
    W = _build_device_weights(reference_angles, mu, kappa, weight)
    bias_col = np.ascontiguousarray(np.asarray(bias, np.float32).reshape(NBINS, 1))
    for c in range(NCORES):
        sl = slice(c * KPC, (c + 1) * KPC)
        xy = np.empty((128, KPC), np.float32)
        xy[:NFREQ] = x[sl].T
        xy[NFREQ:] = y[sl].T
        in_maps.append({"xy": xy, "w": W, "bias": bias_col})

    nc = _get_compiled()
    res = run_bass_kernel_spmd(nc, in_maps, list(range(NCORES)), **spmd_kwargs)

    out = np.empty((NKEYS, NBINS), np.float32)
    for c in range(NCORES):
        out[c * KPC:(c + 1) * KPC] = res.results[c]["out"].T
    return out, res


def kernel(K, reference_angles, mu, kappa, weight, bias):
    out, _ = _run(K, reference_angles, mu, kappa, weight, bias)
    return out



# revision 8
# speedup vs baseline: 1.1636x; 1.1636x over previous
"""Trainium2 kernel for nn_KernelEncodingLayer (von Mises kernel encoding).

Math
----
reference computes, per key n and bin b:
    logits[n,b] = sum_f mag[n,f] * sum_k w[b,f,k] * exp(kappa*(cos(angle[n,f]-mu_eff[b,f,k])-1))

The von Mises kernel expands exactly in a Fourier series (Bessel coefficients):
    exp(kappa*cos(d))*exp(-kappa) = e^-kappa * [I_0(kappa) + 2*sum_m I_m(kappa) cos(m d)]
kappa <= 1 so the series converges superexponentially; truncating cos at m<=2
and sin at m<=3 leaves ~7e-3 max relative error (gate is 2e-2).

With r = mag, u = cos(angle) = x/r, y = r*sin(angle), the needed features are
p_j = r*u^j and q_j = y*u^j, folded with host-side Chebyshev/Bessel math into
per-(bin,freq) weights.  Device chunk layout (contraction rows = 128
partitions; top 64 = p-feature per freq, bottom 64 = q-feature per freq):
    chunk0 = [x ; y  ]  -> (P1, Q0)
    chunk1 = [xu; yu ]  -> (P2, Q1)
    chunk2 = [r ; yu2]  -> (P0, Q2)

Device kernel (per core, 1024 keys), everything fp16 on the wire, fp32 PSUM:
host ships XY=[x;y], XX=[x;x], YY=[y;y] so the whole elementwise chain runs as
full-128-partition ops with no cross-partition copies:
    sq=XX*XX (V)   syb=YY*YY (A square)   r2=sq+syb (V)
    rf=sqrt(r2+eps) (A) = [r;r]           ir=1/rf (V)    uf=XX*ir=[u;u] (V)
    W1=XY*uf=[xu;yu] (V)  W2.bot=W1.bot*uf.bot (V)  W2.top=copy rf.top (A)
then a [128 x key-half] @ [128 x 128bins] PE matmul per chunk accumulated in
PSUM (2 banks of 512 keys), evicted fp16 (bias is added on host), DMA'd out.

Sharding: data-parallel over keys across 8 cores; weights replicated.
"""

import math

import numpy as np

import concourse.bacc as bacc
import concourse.bass as bass
import concourse.mybir as mybir
import concourse.tile as tile
from concourse._compat import with_exitstack
from concourse.bass_utils import run_bass_kernel_spmd
from concourse.mybir import AluOpType

# problem shape (hardcoded per harness contract)
NKEYS = 8192
NBINS = 128
NFREQ = 64
NCORES = 8
KPC = NKEYS // NCORES  # 1024 keys per core
NCHUNK = 3  # contraction chunks: cos harmonics m<=2, sin m<=3
NSPLIT = 2  # key blocks per core for pipelining (PSUM bank per block)
BLK = KPC // NSPLIT

F16 = mybir.dt.float16
F32 = mybir.dt.float32
EPS_GUARD = 1e-6  # r2 guard so 1/r stays bounded

AFT = mybir.ActivationFunctionType


# ----------------------------------------------------------------------------
# host-side math: Bessel I_m and Chebyshev coefficient folding
# ----------------------------------------------------------------------------

def _bessel_i(m: int, x: np.ndarray) -> np.ndarray:
    x = np.asarray(x, np.float64)
    s = np.zeros_like(x)
    for j in range(24):
        s = s + (x / 2.0) ** (2 * j + m) / (math.factorial(j) * math.factorial(j + m))
    return s


def _cheb_t(m: int) -> np.ndarray:
    T = [np.array([1.0]), np.array([0.0, 1.0])]
    while len(T) <= m:
        a = np.zeros(len(T[-1]) + 1)
        a[1:] = 2 * T[-1]
        a[: len(T[-2])] -= T[-2]
        T.append(a)
    return T[m]


def _cheb_u(m: int) -> np.ndarray:
    U = [np.array([1.0]), np.array([0.0, 2.0])]
    while len(U) <= m:
        a = np.zeros(len(U[-1]) + 1)
        a[1:] = 2 * U[-1]
        a[: len(U[-2])] -= U[-2]
        U.append(a)
    return U[m]


def _build_device_weights(reference_angles, mu, kappa, weight) -> np.ndarray:
    """Fold per-(bin,freq) coefficients into [128, NCHUNK*NBINS] fp16.

    Column block c holds chunk c's weights; rows 0:64 multiply the p-feature,
    rows 64:128 the q-feature of that chunk.
    """
    mc, ms = 2, 3  # cos harmonics m<=mc, sin m<=ms
    mu_eff = np.asarray(mu, np.float64) + np.asarray(reference_angles, np.float64)[None, :, None]
    kap = np.asarray(kappa, np.float64)
    w = np.asarray(weight, np.float64)

    P = np.zeros((mc + 1, NBINS, NFREQ))  # coeff of p_j = r*u^j
    Q = np.zeros((ms, NBINS, NFREQ))      # coeff of q_j = y*u^j
    for m in range(0, mc + 1):
        eps = 1.0 if m == 0 else 2.0
        coef = w * eps * _bessel_i(m, kap) * np.exp(-kap)
        A = (coef * np.cos(m * mu_eff)).sum(-1)  # (b, f)
        for j, c in enumerate(_cheb_t(m)):
            if c:
                P[j] += c * A
    for m in range(1, ms + 1):
        coef = w * 2.0 * _bessel_i(m, kap) * np.exp(-kap)
        B = (coef * np.sin(m * mu_eff)).sum(-1)
        for j, c in enumerate(_cheb_u(m - 1)):
            if c:
                Q[j] += c * B

    W = np.zeros((128, NCHUNK * NBINS), np.float64)
    pairs = [(P[1], Q[0]), (P[2], Q[1]), (P[0], Q[2])]
    for c, (top, bot) in enumerate(pairs):
        W[:NFREQ, c * NBINS:(c + 1) * NBINS] = top.T  # (f, b)
        W[NFREQ:, c * NBINS:(c + 1) * NBINS] = bot.T
    return np.ascontiguousarray(W.astype(np.float16))


# ----------------------------------------------------------------------------
# device kernel
# ----------------------------------------------------------------------------

@with_exitstack
def _device_kernel(ctx, tc: tile.TileContext, out_d, xy_d, xx_d, yy_d, w_d):
    nc = tc.nc
    const = ctx.enter_context(tc.tile_pool(name="const", bufs=1))
    work = ctx.enter_context(tc.tile_pool(name="work", bufs=1))
    psum = ctx.enter_context(tc.tile_pool(name="psum", bufs=1, space="PSUM"))

    # eps doubles as the r2 guard bias for sqrt and as the operand of a tiny
    # warm-up op that pulls the ACT table load into the DMA-fill window
    eps = const.tile([128, 1], F32, tag="eps")
    warm = const.tile([128, 1], F32, tag="warm")
    nc.gpsimd.memset(eps[:], EPS_GUARD)
    nc.scalar.sqrt(warm[:], eps[:])

    xy = const.tile([128, KPC], F16, tag="xy")
    xx = const.tile([128, KPC], F16, tag="xx")
    yy = const.tile([128, KPC], F16, tag="yy")
    wt = const.tile([128, NCHUNK * NBINS], F16, tag="wt")
    nc.sync.dma_start(xx[:], xx_d[:])
    nc.sync.dma_start(yy[:], yy_d[:])
    nc.sync.dma_start(xy[:], xy_d[:])
    nc.sync.dma_start(wt[:], w_d[:])

    sq = work.tile([128, KPC], F16, tag="sq")
    syb = work.tile([128, KPC], F16, tag="syb")
    r2 = work.tile([128, KPC], F16, tag="r2")
    rf = work.tile([128, KPC], F16, tag="rf")
    ir = work.tile([128, KPC], F16, tag="ir")
    uf = work.tile([128, KPC], F16, tag="uf")
    w1 = work.tile([128, KPC], F16, tag="w1")
    w2 = work.tile([128, KPC], F16, tag="w2")
    outt = work.tile([128, KPC], F16, tag="outt")

    HF = NFREQ
    ps = [psum.tile([128, BLK], F32, tag=f"ps{h}", name=f"ps{h}") for h in range(NSPLIT)]

    def blk(t, h):
        return t[:, h * BLK:(h + 1) * BLK]

    def blkb(t, h):  # bottom half of a block
        return t[HF:, h * BLK:(h + 1) * BLK]

    with nc.allow_low_precision(reason="fp16 feature chain; validated vs fp64 host sim"):
        for h in range(NSPLIT):
            # chunk0 matmul only needs xy + weights; runs during the chain
            nc.tensor.matmul(ps[h][:], wt[:, 0:NBINS], blk(xy, h), start=True, stop=False)

            nc.vector.tensor_tensor(blk(sq, h), blk(xx, h), blk(xx, h), AluOpType.mult)
            nc.scalar.square(blk(syb, h), blk(yy, h))
            nc.vector.tensor_tensor(blk(r2, h), blk(sq, h), blk(syb, h), AluOpType.add)
            nc.scalar.activation(blk(rf, h), blk(r2, h), AFT.Sqrt, bias=eps[:])
            nc.vector.reciprocal(blk(ir, h), blk(rf, h))
            nc.vector.tensor_tensor(blk(uf, h), blk(xx, h), blk(ir, h), AluOpType.mult)
            nc.vector.tensor_tensor(blk(w1, h), blk(xy, h), blk(uf, h), AluOpType.mult)
            nc.tensor.matmul(ps[h][:], wt[:, NBINS:2 * NBINS], blk(w1, h), start=False, stop=False)
            nc.vector.tensor_tensor(blkb(w2, h), blkb(w1, h), blkb(uf, h), AluOpType.mult)
            nc.scalar.copy(w2[:HF, h * BLK:(h + 1) * BLK], rf[:HF, h * BLK:(h + 1) * BLK])
            nc.tensor.matmul(ps[h][:], wt[:, 2 * NBINS:3 * NBINS], blk(w2, h), start=False, stop=True)

            # evict PSUM -> SBUF fp16 (bias added on host), then DMA out
            if h % 2 == 0:
                nc.scalar.copy(blk(outt, h), ps[h][:])
            else:
                nc.vector.tensor_scalar_add(blk(outt, h), ps[h][:], 0.0)
            nc.sync.dma_start(out_d[:, h * BLK:(h + 1) * BLK], blk(outt, h))


_COMPILED = None


def _get_compiled():
    global _COMPILED
    if _COMPILED is None:
        nc = bacc.Bacc("TRN2", target_bir_lowering=False, debug=False)
        xy = nc.dram_tensor("xy", [128, KPC], F16, kind="ExternalInput").ap()
        xx = nc.dram_tensor("xx", [128, KPC], F16, kind="ExternalInput").ap()
        yy = nc.dram_tensor("yy", [128, KPC], F16, kind="ExternalInput").ap()
        w = nc.dram_tensor("w", [128, NCHUNK * NBINS], F16, kind="ExternalInput").ap()
        out = nc.dram_tensor("out", [NBINS, KPC], F16, kind="ExternalOutput").ap()
        with tile.TileContext(nc) as tc:
            _device_kernel(tc, out, xy, xx, yy, w)
        nc.compile()
        _COMPILED = nc
    return _COMPILED


# ----------------------------------------------------------------------------
# entry point
# ----------------------------------------------------------------------------

def _run(K, reference_angles, mu, kappa, weight, bias, **spmd_kwargs):
    K = np.ascontiguousarray(np.asarray(K, np.float32))
    x = K[:, 0::2].astype(np.float16)  # (NKEYS, NFREQ) real parts
    y = K[:, 1::2].astype(np.float16)  # imag parts

    W = _build_device_weights(reference_angles, mu, kappa, weight)
    in_maps = []
    for c in range(NCORES):
        sl = slice(c * KPC, (c + 1) * KPC)
        xt = np.ascontiguousarray(x[sl].T)  # (64, KPC)
        yt = np.ascontiguousarray(y[sl].T)
        xy = np.empty((128, KPC), np.float16)
        xy[:NFREQ] = xt
        xy[NFREQ:] = yt
        xx = np.empty((128, KPC), np.float16)
        xx[:NFREQ] = xt
        xx[NFREQ:] = xt
        yyt = np.empty((128, KPC), np.float16)
        yyt[:NFREQ] = yt
        yyt[NFREQ:] = yt
        in_maps.append({"xy": xy, "xx": xx, "yy": yyt, "w": W})

    nc = _get_compiled()
    res = run_bass_kernel_spmd(nc, in_maps, list(range(NCORES)), **spmd_kwargs)

    bias32 = np.asarray(bias, np.float32)
    out = np.empty((NKEYS, NBINS), np.float32)
    for c in range(NCORES):
        out[c * KPC:(c + 1) * KPC] = res.results[c]["out"].T.astype(np.float32)
    out += bias32[None, :]
    return out, res


def kernel(K, reference_angles, mu, kappa, weight, bias):
    out, _ = _run(K, reference_angles, mu, kappa, weight, bias)
    return out
